# revision 8
# baseline (speedup 1.0000x reference)
"""CRF loss (forward-algorithm logsumexp recurrence) on 8 NeuronCores.

v3: meet-in-the-middle. The forward score sum_k alpha_{Tb}[k] factors as
P_m^T E S_{n-1} where P is the forward exp-domain state chain
    P_i = (E^T P_{i-1}) o F_i,          P_0 = F_0
and S is an independent *backward* chain over reversed, per-column
time indices
    S_i = (E S_{i-1}) o F_back_i,       S_0 = F_back_0,
with F_back[s][:, b] = F[Tb - s][:, b] (host-gathered per column).
Both chains advance one step per period and are phase-offset on the
same core, so each fills the other's PE->DVE->PE latency; sequential
depth halves to ceil(max_len/2) periods. Every period both chain
states ([128, 2*16] fp16, k-halves as column blocks) are DMAed out;
the host combines P_{m_b} and S_{Tb-m_b-1} per column in float64.
"""

import numpy as np

B, T, K = 128, 256, 256
N_CORES = 8
BL = B // N_CORES   # batch per core (16)
KT = K // 128       # k tiles (2)
GW = KT * BL        # state cols per chain per period (32)
CHUNK = 16          # periods of F per DMA chunk

_cache = {}


def _build_nc(tp):
    from contextlib import ExitStack

    import concourse.bacc as bacc
    import concourse.tile as tile
    from concourse import mybir

    nc = bacc.Bacc("TRN2", target_bir_lowering=False, debug=False,
                   enable_asserts=False, num_devices=N_CORES)
    f16 = mybir.dt.float16

    # 8 weight tiles: 4 for E^T-apply (fwd), 4 for E-apply (bwd)
    e_in = nc.dram_tensor("e_in", [128, 2 * KT * KT * 128], f16,
                          kind="ExternalInput").ap()
    # F cols per period: (chain, j, b) = 2*GW
    f_in = nc.dram_tensor("f_in", [128, tp * 2 * GW], f16,
                          kind="ExternalInput").ap()
    p_out = nc.dram_tensor("p_out", [128, tp * GW], f16,
                           kind="ExternalOutput").ap()
    q_out = nc.dram_tensor("q_out", [128, tp * GW], f16,
                           kind="ExternalOutput").ap()

    WIN = 8          # periods per state-buffer window
    with tile.TileContext(nc) as tc, ExitStack() as ctx:
        consts = ctx.enter_context(tc.tile_pool(name="consts", bufs=1))
        fpool = ctx.enter_context(tc.tile_pool(name="fpool", bufs=3))
        state = ctx.enter_context(tc.tile_pool(name="state", bufs=2))
        psum = ctx.enter_context(tc.tile_pool(name="psum", bufs=3,
                                              space="PSUM"))

        chunk_lens = [3, CHUNK - 3] + [CHUNK] * ((tp - CHUNK) // CHUNK)
        rem = tp - sum(chunk_lens)
        if rem:
            chunk_lens.append(rem)
        chunk_t0 = np.cumsum([0] + chunk_lens).tolist()
        n_chunks = len(chunk_lens)
        step_chunk = {}
        for ci, (t0c, ln) in enumerate(zip(chunk_t0, chunk_lens)):
            for tt_ in range(t0c, t0c + ln):
                step_chunk[tt_] = (ci, tt_ - t0c)
        fch = [None] * n_chunks

        def load_chunk(c, split=1, eng=None):
            ln = chunk_lens[c]
            ft = fpool.tile([128, CHUNK * 2 * GW], f16, tag="f", name="fch")
            w = ln * 2 * GW
            base = chunk_t0[c] * 2 * GW
            if eng is None:
                eng = nc.scalar
            for s in range(split):
                lo, hi = s * w // split, (s + 1) * w // split
                eng.dma_start(ft[:, lo:hi], f_in[:, base + lo:base + hi])
            fch[c] = ft

        # dedicated DMA queues: F chunks alone on scalar (so period 0's
        # copy is gated only by chunk 0), fwd weights + p_out on sync,
        # bwd weights + q_out on gpsimd. Separate lo/hi weight tiles
        # keep the first LDWs gated only on their own DMA.
        load_chunk(0)
        h = KT * KT * 64
        e_tl = {}
        for ch, half, eng in ((0, 0, nc.sync), (0, 1, nc.scalar),
                              (1, 0, nc.sync), (1, 1, nc.scalar)):
            tile_ = consts.tile([128, h], f16, tag=f"e{ch}{half}",
                                name=f"e{ch}{half}")
            eng.dma_start(tile_[:], e_in[:, (2 * ch + half) * h:
                                         (2 * ch + half + 1) * h])
            e_tl[(ch, half)] = tile_
        # e_t[chain][i][j]; packing order per chain: E00,E10 then E01,E11
        e_t = [[[e_tl[(ch, j)][:, i * 128:(i + 1) * 128]
                 for j in range(KT)] for i in range(KT)] for ch in range(2)]

        # chunk 1 on sync (3rd there) so t=3..15 aren't gated behind the
        # scalar queue's weight DMAs
        load_chunk(1, eng=nc.sync)

        outs = [p_out, q_out]
        pbuf_prev = [None, None]
        pbuf = [state.tile([128, WIN * GW], f16, tag=f"pb{ch}",
                           name=f"pb{ch}") for ch in range(2)]
        DMA_GRP = 4      # periods of state per output DMA
        for t in range(tp):
            c, r = step_chunk[t]
            if r == 0 and c + 2 < n_chunks:
                load_chunk(c + 2)
            w, slot = divmod(t, WIN)
            for ch in range(2):
                fcol = fch[c][:, r * 2 * GW + ch * GW:
                              r * 2 * GW + (ch + 1) * GW]   # [128, 32]
                p_new = pbuf[ch][:, slot * GW:(slot + 1) * GW]
                if t == 0:
                    nc.vector.tensor_copy(p_new, fcol)
                else:
                    pslot = (t - 1) % WIN
                    src = pbuf_prev[ch] if pslot == WIN - 1 else pbuf[ch]
                    p_prev = src[:, pslot * GW:(pslot + 1) * GW]
                    ps = psum.tile([128, GW], mybir.dt.float32,
                                   tag=f"ps{ch}", name=f"ps{ch}")
                    et = e_t[ch]
                    nc.tensor.matmul(ps[:, 0:BL], et[0][0][:],
                                     p_prev[:, 0:BL], start=True, stop=False)
                    nc.tensor.matmul(ps[:, 0:BL], et[1][0][:],
                                     p_prev[:, BL:GW], start=False, stop=True)
                    nc.tensor.matmul(ps[:, BL:GW], et[0][1][:],
                                     p_prev[:, 0:BL], start=True, stop=False)
                    nc.tensor.matmul(ps[:, BL:GW], et[1][1][:],
                                     p_prev[:, BL:GW], start=False, stop=True)
                    nc.vector.tensor_mul(p_new[:], ps[:], fcol[:])
                # state out: batched every DMA_GRP periods
                if slot % DMA_GRP == DMA_GRP - 1 or t == tp - 1:
                    g0 = (slot // DMA_GRP) * DMA_GRP
                    lo = (w * WIN + g0) * GW
                    ncols = (slot - g0 + 1) * GW
                    eng = nc.sync if ch == 0 else nc.scalar
                    eng.dma_start(
                        outs[ch][:, lo:lo + ncols],
                        pbuf[ch][:, g0 * GW:g0 * GW + ncols])
            if slot == WIN - 1:
                for ch in range(2):
                    pbuf_prev[ch] = pbuf[ch]
                    pbuf[ch] = state.tile([128, WIN * GW], f16,
                                          tag=f"pb{ch}", name=f"pb{ch}")

    nc.compile()
    return nc


def _prepare(feats, transitions, feats_len, tp):
    f = np.ascontiguousarray(feats.transpose(1, 0, 2)).astype(np.float32)
    # per-step normalizer: mean over batch of logsumexp_k of the frame
    m = f.max(axis=2)
    lse = np.log(np.exp(f - m[:, :, None]).sum(axis=2,
                                               dtype=np.float32)) + m
    c = lse.mean(axis=1).astype(np.float32)             # [T]
    offs = np.cumsum(c.astype(np.float64))              # [T]

    E = np.exp(transitions.astype(np.float32))
    # col layout: (chain, j, i) blocks of 128 — [E00, E10, E01, E11]
    e_packed = np.empty((128, 2 * KT * KT * 128), np.float16)
    for i in range(KT):
        for j in range(KT):
            blk = E[128 * i:128 * (i + 1), 128 * j:128 * (j + 1)]
            e_packed[:, (j * KT + i) * 128:(j * KT + i + 1) * 128] = blk
            # bwd chain applies E (not E^T): stationary lhsT = E^T tiles
            e_packed[:, (KT * KT + j * KT + i) * 128:
                     (KT * KT + j * KT + i + 1) * 128] = \
                E[128 * j:128 * (j + 1), 128 * i:128 * (i + 1)].T
    Fx = np.exp(f - c[:, None, None]).astype(np.float32)  # [T, B, K]

    Tb = (feats_len - 1).astype(np.int64)                 # [B]
    f_maps = []
    for core in range(N_CORES):
        bg = np.arange(core * BL, (core + 1) * BL)
        fwd = Fx[:tp, bg, :]                              # [tp, BL, K]
        rev = Tb[bg][None, :] - np.arange(tp)[:, None]    # [tp, BL]
        bwd = Fx[np.maximum(rev, 0), bg[None, :], :]      # [tp, BL, K]
        # zero frames past each column's start: dead columns decay to 0
        # instead of overflowing fp16 (they are never read by the host)
        bwd[rev < 0] = 0.0
        # layout [128, tp, chain, j, b]
        arr = np.stack([fwd, bwd], axis=1)                # [tp, 2, BL, K]
        blk = arr.reshape(tp, 2, BL, KT, 128).transpose(4, 0, 1, 3, 2)
        f_maps.append({"f_in": np.ascontiguousarray(
            blk.reshape(128, tp * 2 * GW)).astype(np.float16)})
    return e_packed, f_maps, offs, c


def _gold_score(feats, transitions, tags, feats_len):
    f = feats.transpose(1, 0, 2).astype(np.float32)       # [T, B, K]
    tg = tags.T.astype(np.int64)                          # [T, B]
    mask = (np.arange(T)[:, None] < feats_len[None, :])
    maskf = mask.astype(np.float32)
    emit = np.take_along_axis(f, tg[:, :, None], axis=2)[:, :, 0] * maskf
    u = emit.sum(axis=0, dtype=np.float32)
    t_mask = maskf[:-1] * maskf[1:]
    t_score = transitions.astype(np.float32)[tg[:-1], tg[1:]] * t_mask
    return (u + t_score.sum(axis=0, dtype=np.float32)).astype(np.float32)


def kernel(feats, transitions, tags, feats_len, _results_hook=None,
           _trace=False):
    from concourse.bass_utils import run_bass_kernel_spmd

    feats = np.asarray(feats, dtype=np.float32)
    transitions = np.asarray(transitions, dtype=np.float32)
    tags_np = np.asarray(tags)
    feats_len_np = np.asarray(feats_len).astype(np.int64)

    max_len = int(feats_len_np.max())
    tp = min(T, max((max_len - 1) // 2 + 1, 2 * CHUNK))
    if ("nc", tp) not in _cache:
        _cache[("nc", tp)] = _build_nc(tp)
    nc = _cache[("nc", tp)]

    e_packed, f_maps, offs, _c = _prepare(feats, transitions,
                                          feats_len_np, tp)
    in_maps = [{"e_in": e_packed, **f_maps[core]} for core in range(N_CORES)]

    res = run_bass_kernel_spmd(nc, in_maps, core_ids=list(range(N_CORES)),
                               trace=_trace)
    if _results_hook is not None:
        _results_hook(res)

    u = _gold_score(feats, transitions, tags_np, feats_len_np)
    E64 = np.exp(transitions.astype(np.float64))
    Tb = feats_len_np - 1
    loss = np.empty(B, np.float32)
    for core in range(N_CORES):
        bg = np.arange(core * BL, (core + 1) * BL)
        # [128, tp, j, b] -> [tp, K, b]
        P = res.results[core]["p_out"].reshape(128, tp, KT, BL).astype(
            np.float64).transpose(1, 2, 0, 3).reshape(tp, K, BL)
        S = res.results[core]["q_out"].reshape(128, tp, KT, BL).astype(
            np.float64).transpose(1, 2, 0, 3).reshape(tp, K, BL)
        for i, b in enumerate(bg):
            tb = int(Tb[b])
            if tb == 0:
                d = P[0, :, i].sum()
            else:
                mb = tb // 2
                x = P[mb, :, i]                       # fwd state at mb
                y = S[tb - mb - 1, :, i]              # bwd state
                d = (x @ E64) @ y
            loss[b] = np.float32(np.log(d) + offs[tb] - u[b])
    return loss


# revision 10
# speedup vs baseline: 1.0145x; 1.0145x over previous
"""CRF loss (forward-algorithm logsumexp recurrence) on 8 NeuronCores.

v3: meet-in-the-middle. The forward score sum_k alpha_{Tb}[k] factors as
P_m^T E S_{n-1} where P is the forward exp-domain state chain
    P_i = (E^T P_{i-1}) o F_i,          P_0 = F_0
and S is an independent *backward* chain over reversed, per-column
time indices
    S_i = (E S_{i-1}) o F_back_i,       S_0 = F_back_0,
with F_back[s][:, b] = F[Tb - s][:, b] (host-gathered per column).
Both chains advance one step per period and are phase-offset on the
same core, so each fills the other's PE->DVE->PE latency; sequential
depth halves to ceil(max_len/2) periods. Every period both chain
states ([128, 2*16] fp16, k-halves as column blocks) are DMAed out;
the host combines P_{m_b} and S_{Tb-m_b-1} per column in float64.
"""

import numpy as np

B, T, K = 128, 256, 256
N_CORES = 8
BL = B // N_CORES   # batch per core (16)
KT = K // 128       # k tiles (2)
GW = KT * BL        # state cols per chain per period (32)
CHUNK = 16          # periods of F per DMA chunk

_cache = {}


def _build_nc(tp):
    from contextlib import ExitStack

    import concourse.bacc as bacc
    import concourse.tile as tile
    from concourse import mybir

    nc = bacc.Bacc("TRN2", target_bir_lowering=False, debug=False,
                   enable_asserts=False, num_devices=N_CORES)
    f16 = mybir.dt.float16

    # 8 weight tiles: 4 for E^T-apply (fwd), 4 for E-apply (bwd)
    e_in = nc.dram_tensor("e_in", [128, 2 * KT * KT * 128], f16,
                          kind="ExternalInput").ap()
    # F cols per period: (chain, j, b) = 2*GW
    f_in = nc.dram_tensor("f_in", [128, tp * 2 * GW], f16,
                          kind="ExternalInput").ap()
    p_out = nc.dram_tensor("p_out", [128, tp * GW], f16,
                           kind="ExternalOutput").ap()
    q_out = nc.dram_tensor("q_out", [128, tp * GW], f16,
                           kind="ExternalOutput").ap()

    WIN = 8          # periods per state-buffer window
    with tile.TileContext(nc) as tc, ExitStack() as ctx:
        consts = ctx.enter_context(tc.tile_pool(name="consts", bufs=1))
        fpool = ctx.enter_context(tc.tile_pool(name="fpool", bufs=3))
        state = ctx.enter_context(tc.tile_pool(name="state", bufs=2))
        psum = ctx.enter_context(tc.tile_pool(name="psum", bufs=3,
                                              space="PSUM"))

        chunk_lens = [6, CHUNK - 6] + [CHUNK] * ((tp - CHUNK) // CHUNK)
        rem = tp - sum(chunk_lens)
        if rem:
            chunk_lens.append(rem)
        chunk_t0 = np.cumsum([0] + chunk_lens).tolist()
        n_chunks = len(chunk_lens)
        step_chunk = {}
        for ci, (t0c, ln) in enumerate(zip(chunk_t0, chunk_lens)):
            for tt_ in range(t0c, t0c + ln):
                step_chunk[tt_] = (ci, tt_ - t0c)
        fch = [None] * n_chunks

        def load_chunk(c, split=1, eng=None):
            ln = chunk_lens[c]
            ft = fpool.tile([128, CHUNK * 2 * GW], f16, tag="f", name="fch")
            w = ln * 2 * GW
            base = chunk_t0[c] * 2 * GW
            if eng is None:
                eng = nc.scalar
            for s in range(split):
                lo, hi = s * w // split, (s + 1) * w // split
                eng.dma_start(ft[:, lo:hi], f_in[:, base + lo:base + hi])
            fch[c] = ft

        # startup: every DMA-fed tile pays ~2us completion-sem latency on
        # first use, so pack all early needs into each queue's FIRST DMA:
        # all 8 weight tiles in one sync DMA, a 6-period chunk 0 as the
        # first scalar DMA. p_out rides sync, q_out + later chunks ride
        # scalar; gpsimd (software DGE, slow drain) carries nothing.
        load_chunk(0)
        h = KT * KT * 64
        e_all = consts.tile([128, 4 * h], f16, tag="eall", name="eall")
        nc.sync.dma_start(e_all[:], e_in[:])
        # e_t[chain][i][j]; packing: (chain, j, i) blocks of 128 cols
        e_t = [[[e_all[:, ((2 * ch + j) * KT + i) * 128:
                       ((2 * ch + j) * KT + i + 1) * 128]
                 for j in range(KT)] for i in range(KT)] for ch in range(2)]

        load_chunk(1)

        outs = [p_out, q_out]
        pbuf_prev = [None, None]
        pbuf = [state.tile([128, WIN * GW], f16, tag=f"pb{ch}",
                           name=f"pb{ch}") for ch in range(2)]
        DMA_GRP = 4      # periods of state per output DMA
        for t in range(tp):
            c, r = step_chunk[t]
            if r == 0 and c + 2 < n_chunks:
                load_chunk(c + 2)
            w, slot = divmod(t, WIN)
            for ch in range(2):
                fcol = fch[c][:, r * 2 * GW + ch * GW:
                              r * 2 * GW + (ch + 1) * GW]   # [128, 32]
                p_new = pbuf[ch][:, slot * GW:(slot + 1) * GW]
                if t == 0:
                    nc.vector.tensor_copy(p_new, fcol)
                else:
                    pslot = (t - 1) % WIN
                    src = pbuf_prev[ch] if pslot == WIN - 1 else pbuf[ch]
                    p_prev = src[:, pslot * GW:(pslot + 1) * GW]
                    ps = psum.tile([128, GW], mybir.dt.float32,
                                   tag=f"ps{ch}", name=f"ps{ch}")
                    et = e_t[ch]
                    nc.tensor.matmul(ps[:, 0:BL], et[0][0][:],
                                     p_prev[:, 0:BL], start=True, stop=False)
                    nc.tensor.matmul(ps[:, 0:BL], et[1][0][:],
                                     p_prev[:, BL:GW], start=False, stop=True)
                    nc.tensor.matmul(ps[:, BL:GW], et[0][1][:],
                                     p_prev[:, 0:BL], start=True, stop=False)
                    nc.tensor.matmul(ps[:, BL:GW], et[1][1][:],
                                     p_prev[:, BL:GW], start=False, stop=True)
                    nc.vector.tensor_mul(p_new[:], ps[:], fcol[:])
                # state out: batched every DMA_GRP periods
                if slot % DMA_GRP == DMA_GRP - 1 or t == tp - 1:
                    g0 = (slot // DMA_GRP) * DMA_GRP
                    lo = (w * WIN + g0) * GW
                    ncols = (slot - g0 + 1) * GW
                    eng = nc.sync if ch == 0 else nc.scalar
                    eng.dma_start(
                        outs[ch][:, lo:lo + ncols],
                        pbuf[ch][:, g0 * GW:g0 * GW + ncols])
            if slot == WIN - 1:
                for ch in range(2):
                    pbuf_prev[ch] = pbuf[ch]
                    pbuf[ch] = state.tile([128, WIN * GW], f16,
                                          tag=f"pb{ch}", name=f"pb{ch}")

    nc.compile()
    return nc


def _prepare(feats, transitions, feats_len, tp):
    f = np.ascontiguousarray(feats.transpose(1, 0, 2)).astype(np.float32)
    # per-step normalizer: mean over batch of logsumexp_k of the frame
    m = f.max(axis=2)
    lse = np.log(np.exp(f - m[:, :, None]).sum(axis=2,
                                               dtype=np.float32)) + m
    c = lse.mean(axis=1).astype(np.float32)             # [T]
    offs = np.cumsum(c.astype(np.float64))              # [T]

    E = np.exp(transitions.astype(np.float32))
    # col layout: (chain, j, i) blocks of 128 — [E00, E10, E01, E11]
    e_packed = np.empty((128, 2 * KT * KT * 128), np.float16)
    for i in range(KT):
        for j in range(KT):
            blk = E[128 * i:128 * (i + 1), 128 * j:128 * (j + 1)]
            e_packed[:, (j * KT + i) * 128:(j * KT + i + 1) * 128] = blk
            # bwd chain applies E (not E^T): stationary lhsT = E^T tiles
            e_packed[:, (KT * KT + j * KT + i) * 128:
                     (KT * KT + j * KT + i + 1) * 128] = \
                E[128 * j:128 * (j + 1), 128 * i:128 * (i + 1)].T
    Fx = np.exp(f - c[:, None, None]).astype(np.float32)  # [T, B, K]

    Tb = (feats_len - 1).astype(np.int64)                 # [B]
    f_maps = []
    for core in range(N_CORES):
        bg = np.arange(core * BL, (core + 1) * BL)
        fwd = Fx[:tp, bg, :]                              # [tp, BL, K]
        rev = Tb[bg][None, :] - np.arange(tp)[:, None]    # [tp, BL]
        bwd = Fx[np.maximum(rev, 0), bg[None, :], :]      # [tp, BL, K]
        # zero frames past each column's start: dead columns decay to 0
        # instead of overflowing fp16 (they are never read by the host)
        bwd[rev < 0] = 0.0
        # layout [128, tp, chain, j, b]
        arr = np.stack([fwd, bwd], axis=1)                # [tp, 2, BL, K]
        blk = arr.reshape(tp, 2, BL, KT, 128).transpose(4, 0, 1, 3, 2)
        f_maps.append({"f_in": np.ascontiguousarray(
            blk.reshape(128, tp * 2 * GW)).astype(np.float16)})
    return e_packed, f_maps, offs, c


def _gold_score(feats, transitions, tags, feats_len):
    f = feats.transpose(1, 0, 2).astype(np.float32)       # [T, B, K]
    tg = tags.T.astype(np.int64)                          # [T, B]
    mask = (np.arange(T)[:, None] < feats_len[None, :])
    maskf = mask.astype(np.float32)
    emit = np.take_along_axis(f, tg[:, :, None], axis=2)[:, :, 0] * maskf
    u = emit.sum(axis=0, dtype=np.float32)
    t_mask = maskf[:-1] * maskf[1:]
    t_score = transitions.astype(np.float32)[tg[:-1], tg[1:]] * t_mask
    return (u + t_score.sum(axis=0, dtype=np.float32)).astype(np.float32)


def kernel(feats, transitions, tags, feats_len, _results_hook=None,
           _trace=False):
    from concourse.bass_utils import run_bass_kernel_spmd

    feats = np.asarray(feats, dtype=np.float32)
    transitions = np.asarray(transitions, dtype=np.float32)
    tags_np = np.asarray(tags)
    feats_len_np = np.asarray(feats_len).astype(np.int64)

    max_len = int(feats_len_np.max())
    tp = min(T, max((max_len - 1) // 2 + 1, 2 * CHUNK))
    if ("nc", tp) not in _cache:
        _cache[("nc", tp)] = _build_nc(tp)
    nc = _cache[("nc", tp)]

    e_packed, f_maps, offs, _c = _prepare(feats, transitions,
                                          feats_len_np, tp)
    in_maps = [{"e_in": e_packed, **f_maps[core]} for core in range(N_CORES)]

    res = run_bass_kernel_spmd(nc, in_maps, core_ids=list(range(N_CORES)),
                               trace=_trace)
    if _results_hook is not None:
        _results_hook(res)

    u = _gold_score(feats, transitions, tags_np, feats_len_np)
    E64 = np.exp(transitions.astype(np.float64))
    Tb = feats_len_np - 1
    loss = np.empty(B, np.float32)
    for core in range(N_CORES):
        bg = np.arange(core * BL, (core + 1) * BL)
        # [128, tp, j, b] -> [tp, K, b]
        P = res.results[core]["p_out"].reshape(128, tp, KT, BL).astype(
            np.float64).transpose(1, 2, 0, 3).reshape(tp, K, BL)
        S = res.results[core]["q_out"].reshape(128, tp, KT, BL).astype(
            np.float64).transpose(1, 2, 0, 3).reshape(tp, K, BL)
        for i, b in enumerate(bg):
            tb = int(Tb[b])
            if tb == 0:
                d = P[0, :, i].sum()
            else:
                mb = tb // 2
                x = P[mb, :, i]                       # fwd state at mb
                y = S[tb - mb - 1, :, i]              # bwd state
                d = (x @ E64) @ y
            loss[b] = np.float32(np.log(d) + offs[tb] - u[b])
    return loss


# revision 13
# speedup vs baseline: 1.0210x; 1.0063x over previous
"""CRF loss (forward-algorithm logsumexp recurrence) on 8 NeuronCores.

v3: meet-in-the-middle. The forward score sum_k alpha_{Tb}[k] factors as
P_m^T E S_{n-1} where P is the forward exp-domain state chain
    P_i = (E^T P_{i-1}) o F_i,          P_0 = F_0
and S is an independent *backward* chain over reversed, per-column
time indices
    S_i = (E S_{i-1}) o F_back_i,       S_0 = F_back_0,
with F_back[s][:, b] = F[Tb - s][:, b] (host-gathered per column).
Both chains advance one step per period and are phase-offset on the
same core, so each fills the other's PE->DVE->PE latency; sequential
depth halves to ceil(max_len/2) periods. Every period both chain
states ([128, 2*16] fp16, k-halves as column blocks) are DMAed out;
the host combines P_{m_b} and S_{Tb-m_b-1} per column in float64.
"""

import numpy as np

B, T, K = 128, 256, 256
N_CORES = 8
BL = B // N_CORES   # batch per core (16)
KT = K // 128       # k tiles (2)
GW = KT * BL        # state cols per chain per period (32)
CHUNK = 16          # periods of F per DMA chunk

_cache = {}


def _build_nc(tp):
    from contextlib import ExitStack

    import concourse.bacc as bacc
    import concourse.tile as tile
    from concourse import mybir

    nc = bacc.Bacc("TRN2", target_bir_lowering=False, debug=False,
                   enable_asserts=False, num_devices=N_CORES)
    f16 = mybir.dt.float16

    # 8 weight tiles: 4 for E^T-apply (fwd), 4 for E-apply (bwd)
    e_in = nc.dram_tensor("e_in", [128, 2 * KT * KT * 128], f16,
                          kind="ExternalInput").ap()
    # F cols per period: (chain, j, b) = 2*GW
    f_in = nc.dram_tensor("f_in", [128, tp * 2 * GW], f16,
                          kind="ExternalInput").ap()
    p_out = nc.dram_tensor("p_out", [128, tp * GW], f16,
                           kind="ExternalOutput").ap()
    q_out = nc.dram_tensor("q_out", [128, tp * GW], f16,
                           kind="ExternalOutput").ap()

    WIN = 8          # periods per state-buffer window
    with tile.TileContext(nc) as tc, ExitStack() as ctx:
        consts = ctx.enter_context(tc.tile_pool(name="consts", bufs=1))
        fpool = ctx.enter_context(tc.tile_pool(name="fpool", bufs=3))
        state = ctx.enter_context(tc.tile_pool(name="state", bufs=2))
        psum = ctx.enter_context(tc.tile_pool(name="psum", bufs=3,
                                              space="PSUM"))

        chunk_lens = [6, CHUNK - 6] + [CHUNK] * ((tp - CHUNK) // CHUNK)
        rem = tp - sum(chunk_lens)
        if rem:
            chunk_lens.append(rem)
        chunk_t0 = np.cumsum([0] + chunk_lens).tolist()
        n_chunks = len(chunk_lens)
        step_chunk = {}
        for ci, (t0c, ln) in enumerate(zip(chunk_t0, chunk_lens)):
            for tt_ in range(t0c, t0c + ln):
                step_chunk[tt_] = (ci, tt_ - t0c)
        fch = [None] * n_chunks

        def load_chunk(c, split=1, eng=None):
            ln = chunk_lens[c]
            ft = fpool.tile([128, CHUNK * 2 * GW], f16, tag="f", name="fch")
            w = ln * 2 * GW
            base = chunk_t0[c] * 2 * GW
            if eng is None:
                eng = nc.scalar
            for s in range(split):
                lo, hi = s * w // split, (s + 1) * w // split
                eng.dma_start(ft[:, lo:hi], f_in[:, base + lo:base + hi])
            fch[c] = ft

        # startup: every DMA-fed tile pays ~2us completion-sem latency on
        # first use, so pack all early needs into each queue's FIRST DMA:
        # all 8 weight tiles in one sync DMA, a 6-period chunk 0 as the
        # first scalar DMA. p_out rides sync, q_out + later chunks ride
        # scalar; gpsimd (software DGE, slow drain) carries nothing.
        load_chunk(0)
        h = KT * KT * 64
        e_all = consts.tile([128, 4 * h], f16, tag="eall", name="eall")
        nc.sync.dma_start(e_all[:], e_in[:])
        # e_t[chain][i][j]; packing: (chain, j, i) blocks of 128 cols
        e_t = [[[e_all[:, ((2 * ch + j) * KT + i) * 128:
                       ((2 * ch + j) * KT + i + 1) * 128]
                 for j in range(KT)] for i in range(KT)] for ch in range(2)]

        load_chunk(1)

        outs = [p_out, q_out]
        pbuf_prev = [None, None]
        pbuf = [state.tile([128, WIN * GW], f16, tag=f"pb{ch}",
                           name=f"pb{ch}") for ch in range(2)]
        DMA_GRP = 4      # periods of state per output DMA
        for t in range(tp):
            c, r = step_chunk[t]
            if r == 0 and c + 2 < n_chunks:
                load_chunk(c + 2)
            w, slot = divmod(t, WIN)
            for ch in range(2):
                fcol = fch[c][:, r * 2 * GW + ch * GW:
                              r * 2 * GW + (ch + 1) * GW]   # [128, 32]
                p_new = pbuf[ch][:, slot * GW:(slot + 1) * GW]
                if t == 0:
                    nc.vector.tensor_copy(p_new, fcol)
                else:
                    pslot = (t - 1) % WIN
                    src = pbuf_prev[ch] if pslot == WIN - 1 else pbuf[ch]
                    p_prev = src[:, pslot * GW:(pslot + 1) * GW]
                    ps = psum.tile([128, GW], mybir.dt.float32,
                                   tag=f"ps{ch}", name=f"ps{ch}")
                    et = e_t[ch]
                    nc.tensor.matmul(ps[:, 0:BL], et[0][0][:],
                                     p_prev[:, 0:BL], start=True, stop=False)
                    nc.tensor.matmul(ps[:, 0:BL], et[1][0][:],
                                     p_prev[:, BL:GW], start=False, stop=True)
                    nc.tensor.matmul(ps[:, BL:GW], et[0][1][:],
                                     p_prev[:, 0:BL], start=True, stop=False)
                    nc.tensor.matmul(ps[:, BL:GW], et[1][1][:],
                                     p_prev[:, BL:GW], start=False, stop=True)
                    nc.vector.tensor_mul(p_new[:], ps[:], fcol[:])
                # state out: batched every DMA_GRP periods
                if slot % DMA_GRP == DMA_GRP - 1 or t == tp - 1:
                    g0 = (slot // DMA_GRP) * DMA_GRP
                    lo = (w * WIN + g0) * GW
                    ncols = (slot - g0 + 1) * GW
                    eng = nc.sync if ch == 0 else nc.scalar
                    eng.dma_start(
                        outs[ch][:, lo:lo + ncols],
                        pbuf[ch][:, g0 * GW:g0 * GW + ncols])
            if slot == WIN - 1:
                for ch in range(2):
                    pbuf_prev[ch] = pbuf[ch]
                    pbuf[ch] = state.tile([128, WIN * GW], f16,
                                          tag=f"pb{ch}", name=f"pb{ch}")

    nc.compile()
    return nc


def _prepare(feats, transitions, feats_len, tp):
    f = np.ascontiguousarray(feats.transpose(1, 0, 2)).astype(np.float32)
    # per-step normalizer: mean over batch of logsumexp_k of the frame
    m = f.max(axis=2)
    lse = np.log(np.exp(f - m[:, :, None]).sum(axis=2,
                                               dtype=np.float32)) + m
    c = lse.mean(axis=1).astype(np.float32)             # [T]
    offs = np.cumsum(c.astype(np.float64))              # [T]

    E = np.exp(transitions.astype(np.float32))
    # col layout: (chain, j, i) blocks of 128 — [E00, E10, E01, E11]
    e_packed = np.empty((128, 2 * KT * KT * 128), np.float16)
    for i in range(KT):
        for j in range(KT):
            blk = E[128 * i:128 * (i + 1), 128 * j:128 * (j + 1)]
            e_packed[:, (j * KT + i) * 128:(j * KT + i + 1) * 128] = blk
            # bwd chain applies E (not E^T): stationary lhsT = E^T tiles
            e_packed[:, (KT * KT + j * KT + i) * 128:
                     (KT * KT + j * KT + i + 1) * 128] = \
                E[128 * j:128 * (j + 1), 128 * i:128 * (i + 1)].T
    Fx = np.exp(f - c[:, None, None]).astype(np.float32)  # [T, B, K]

    Tb = (feats_len - 1).astype(np.int64)                 # [B]
    f_maps = []
    for core in range(N_CORES):
        bg = np.arange(core * BL, (core + 1) * BL)
        fwd = Fx[:tp, bg, :]                              # [tp, BL, K]
        rev = Tb[bg][None, :] - np.arange(tp)[:, None]    # [tp, BL]
        bwd = Fx[np.maximum(rev, 0), bg[None, :], :]      # [tp, BL, K]
        # zero frames past each column's start: dead columns decay to 0
        # instead of overflowing fp16 (they are never read by the host)
        bwd[rev < 0] = 0.0
        # layout [128, tp, chain, j, b]
        arr = np.stack([fwd, bwd], axis=1)                # [tp, 2, BL, K]
        blk = arr.reshape(tp, 2, BL, KT, 128).transpose(4, 0, 1, 3, 2)
        f_maps.append({"f_in": np.ascontiguousarray(
            blk.reshape(128, tp * 2 * GW)).astype(np.float16)})
    return e_packed, f_maps, offs, c, Fx


def _gold_score(feats, transitions, tags, feats_len):
    f = feats.transpose(1, 0, 2).astype(np.float32)       # [T, B, K]
    tg = tags.T.astype(np.int64)                          # [T, B]
    mask = (np.arange(T)[:, None] < feats_len[None, :])
    maskf = mask.astype(np.float32)
    emit = np.take_along_axis(f, tg[:, :, None], axis=2)[:, :, 0] * maskf
    u = emit.sum(axis=0, dtype=np.float32)
    t_mask = maskf[:-1] * maskf[1:]
    t_score = transitions.astype(np.float32)[tg[:-1], tg[1:]] * t_mask
    return (u + t_score.sum(axis=0, dtype=np.float32)).astype(np.float32)


def kernel(feats, transitions, tags, feats_len, _results_hook=None,
           _trace=False):
    from concourse.bass_utils import run_bass_kernel_spmd

    feats = np.asarray(feats, dtype=np.float32)
    transitions = np.asarray(transitions, dtype=np.float32)
    tags_np = np.asarray(tags)
    feats_len_np = np.asarray(feats_len).astype(np.int64)

    # the device loop covers the bulk; the few columns whose meet-point
    # indices land in the last HOST_H periods are stepped forward on the
    # host in float64 (same recurrence, exact frames)
    HOST_H = 2
    max_len = int(feats_len_np.max())
    tp = min(T, max((max_len - 1) // 2 + 1 - HOST_H, 2 * CHUNK))
    if ("nc", tp) not in _cache:
        _cache[("nc", tp)] = _build_nc(tp)
    nc = _cache[("nc", tp)]

    e_packed, f_maps, offs, _c, Fx = _prepare(feats, transitions,
                                              feats_len_np, tp)
    in_maps = [{"e_in": e_packed, **f_maps[core]} for core in range(N_CORES)]

    res = run_bass_kernel_spmd(nc, in_maps, core_ids=list(range(N_CORES)),
                               trace=_trace)
    if _results_hook is not None:
        _results_hook(res)

    u = _gold_score(feats, transitions, tags_np, feats_len_np)
    E64 = np.exp(transitions.astype(np.float64))
    Tb = feats_len_np - 1
    loss = np.empty(B, np.float32)
    for core in range(N_CORES):
        bg = np.arange(core * BL, (core + 1) * BL)
        # [128, tp, j, b] -> [tp, K, b]
        P = res.results[core]["p_out"].reshape(128, tp, KT, BL).astype(
            np.float64).transpose(1, 2, 0, 3).reshape(tp, K, BL)
        S = res.results[core]["q_out"].reshape(128, tp, KT, BL).astype(
            np.float64).transpose(1, 2, 0, 3).reshape(tp, K, BL)
        for i, b in enumerate(bg):
            tb = int(Tb[b])
            if tb == 0:
                d = P[0, :, i].sum()
            else:
                mb = tb // 2
                sidx = tb - mb - 1
                if mb < tp:
                    x = P[mb, :, i]                   # fwd state at mb
                else:
                    x = P[tp - 1, :, i]
                    for t_ in range(tp, mb + 1):
                        x = (E64.T @ x) * Fx[t_, b, :]
                if sidx < tp:
                    y = S[sidx, :, i]                 # bwd state
                else:
                    y = S[tp - 1, :, i]
                    for s_ in range(tp, sidx + 1):
                        y = (E64 @ y) * Fx[tb - s_, b, :]
                d = (x @ E64) @ y
            loss[b] = np.float32(np.log(d) + offs[tb] - u[b])
    return loss


# revision 15
# speedup vs baseline: 1.0277x; 1.0066x over previous
"""CRF loss (forward-algorithm logsumexp recurrence) on 8 NeuronCores.

v3: meet-in-the-middle. The forward score sum_k alpha_{Tb}[k] factors as
P_m^T E S_{n-1} where P is the forward exp-domain state chain
    P_i = (E^T P_{i-1}) o F_i,          P_0 = F_0
and S is an independent *backward* chain over reversed, per-column
time indices
    S_i = (E S_{i-1}) o F_back_i,       S_0 = F_back_0,
with F_back[s][:, b] = F[Tb - s][:, b] (host-gathered per column).
Both chains advance one step per period and are phase-offset on the
same core, so each fills the other's PE->DVE->PE latency; sequential
depth halves to ceil(max_len/2) periods. Every period both chain
states ([128, 2*16] fp16, k-halves as column blocks) are DMAed out;
the host combines P_{m_b} and S_{Tb-m_b-1} per column in float64.
"""

import numpy as np

B, T, K = 128, 256, 256
N_CORES = 8
BL = B // N_CORES   # batch per core (16)
KT = K // 128       # k tiles (2)
GW = KT * BL        # state cols per chain per period (32)
CHUNK = 16          # periods of F per DMA chunk

_cache = {}


def _build_nc(tp):
    from contextlib import ExitStack

    import concourse.bacc as bacc
    import concourse.tile as tile
    from concourse import mybir

    nc = bacc.Bacc("TRN2", target_bir_lowering=False, debug=False,
                   enable_asserts=False, num_devices=N_CORES)
    f16 = mybir.dt.float16

    # 8 weight tiles: 4 for E^T-apply (fwd), 4 for E-apply (bwd)
    e_in = nc.dram_tensor("e_in", [128, 2 * KT * KT * 128], f16,
                          kind="ExternalInput").ap()
    # F cols per period: (chain, j, b) = 2*GW
    f_in = nc.dram_tensor("f_in", [128, tp * 2 * GW], f16,
                          kind="ExternalInput").ap()
    p_out = nc.dram_tensor("p_out", [128, tp * GW], f16,
                           kind="ExternalOutput").ap()
    q_out = nc.dram_tensor("q_out", [128, tp * GW], f16,
                           kind="ExternalOutput").ap()

    WIN = 8          # periods per state-buffer window
    with tile.TileContext(nc) as tc, ExitStack() as ctx:
        consts = ctx.enter_context(tc.tile_pool(name="consts", bufs=1))
        fpool = ctx.enter_context(tc.tile_pool(name="fpool", bufs=3))
        state = ctx.enter_context(tc.tile_pool(name="state", bufs=2))
        psum = ctx.enter_context(tc.tile_pool(name="psum", bufs=3,
                                              space="PSUM"))

        chunk_lens = [6, CHUNK - 6] + [CHUNK] * ((tp - CHUNK) // CHUNK)
        rem = tp - sum(chunk_lens)
        if rem:
            chunk_lens.append(rem)
        chunk_t0 = np.cumsum([0] + chunk_lens).tolist()
        n_chunks = len(chunk_lens)
        step_chunk = {}
        for ci, (t0c, ln) in enumerate(zip(chunk_t0, chunk_lens)):
            for tt_ in range(t0c, t0c + ln):
                step_chunk[tt_] = (ci, tt_ - t0c)
        fch = [None] * n_chunks

        def load_chunk(c, split=1, eng=None):
            ln = chunk_lens[c]
            ft = fpool.tile([128, CHUNK * 2 * GW], f16, tag="f", name="fch")
            w = ln * 2 * GW
            base = chunk_t0[c] * 2 * GW
            if eng is None:
                eng = nc.scalar
            for s in range(split):
                lo, hi = s * w // split, (s + 1) * w // split
                eng.dma_start(ft[:, lo:hi], f_in[:, base + lo:base + hi])
            fch[c] = ft

        # startup: every DMA-fed tile pays ~2us completion-sem latency on
        # first use, so pack all early needs into each queue's FIRST DMA:
        # all 8 weight tiles in one sync DMA, a 6-period chunk 0 as the
        # first scalar DMA. p_out rides sync, q_out + later chunks ride
        # scalar; gpsimd (software DGE, slow drain) carries nothing.
        load_chunk(0)
        h = KT * KT * 64
        e_all = consts.tile([128, 4 * h], f16, tag="eall", name="eall")
        nc.sync.dma_start(e_all[:], e_in[:])
        # e_t[chain][i][j]; packing: (chain, j, i) blocks of 128 cols
        e_t = [[[e_all[:, ((2 * ch + j) * KT + i) * 128:
                       ((2 * ch + j) * KT + i + 1) * 128]
                 for j in range(KT)] for i in range(KT)] for ch in range(2)]

        load_chunk(1)

        outs = [p_out, q_out]
        pbuf_prev = [None, None]
        pbuf = [state.tile([128, WIN * GW], f16, tag=f"pb{ch}",
                           name=f"pb{ch}") for ch in range(2)]
        DMA_GRP = 4      # periods of state per output DMA
        for t in range(tp):
            c, r = step_chunk[t]
            if r == 0 and c + 2 < n_chunks:
                load_chunk(c + 2)
            w, slot = divmod(t, WIN)
            for ch in range(2):
                fcol = fch[c][:, r * 2 * GW + ch * GW:
                              r * 2 * GW + (ch + 1) * GW]   # [128, 32]
                p_new = pbuf[ch][:, slot * GW:(slot + 1) * GW]
                if t == 0:
                    nc.vector.tensor_copy(p_new, fcol)
                else:
                    pslot = (t - 1) % WIN
                    src = pbuf_prev[ch] if pslot == WIN - 1 else pbuf[ch]
                    p_prev = src[:, pslot * GW:(pslot + 1) * GW]
                    ps = psum.tile([128, GW], mybir.dt.float32,
                                   tag=f"ps{ch}", name=f"ps{ch}")
                    et = e_t[ch]
                    nc.tensor.matmul(ps[:, 0:BL], et[0][0][:],
                                     p_prev[:, 0:BL], start=True, stop=False)
                    nc.tensor.matmul(ps[:, 0:BL], et[1][0][:],
                                     p_prev[:, BL:GW], start=False, stop=True)
                    nc.tensor.matmul(ps[:, BL:GW], et[0][1][:],
                                     p_prev[:, 0:BL], start=True, stop=False)
                    nc.tensor.matmul(ps[:, BL:GW], et[1][1][:],
                                     p_prev[:, BL:GW], start=False, stop=True)
                    nc.vector.tensor_mul(p_new[:], ps[:], fcol[:])
                # state out: only slots {0,1} of each 4-period group (the
                # host steps forward from the nearest dumped index), as one
                # [128, 64] DMA per group — halves SBUF-read traffic that
                # otherwise contends with the PE/DVE in-loop
                emit = None
                if slot % 4 == 1:
                    emit = (slot - 1, slot)
                elif t == tp - 1:
                    emit = (slot, slot) if slot % 4 == 0 else \
                        (slot - 1, slot)
                if emit is not None:
                    g0, g1 = emit
                    lo = (w * WIN + g0) * GW
                    ncols = (g1 - g0 + 1) * GW
                    eng = nc.sync if ch == 0 else nc.scalar
                    eng.dma_start(
                        outs[ch][:, lo:lo + ncols],
                        pbuf[ch][:, g0 * GW:g0 * GW + ncols])
            if slot == WIN - 1:
                for ch in range(2):
                    pbuf_prev[ch] = pbuf[ch]
                    pbuf[ch] = state.tile([128, WIN * GW], f16,
                                          tag=f"pb{ch}", name=f"pb{ch}")

    nc.compile()
    return nc


def _prepare(feats, transitions, feats_len, tp):
    f = np.ascontiguousarray(feats.transpose(1, 0, 2)).astype(np.float32)
    # per-step normalizer: mean over batch of logsumexp_k of the frame
    m = f.max(axis=2)
    lse = np.log(np.exp(f - m[:, :, None]).sum(axis=2,
                                               dtype=np.float32)) + m
    c = lse.mean(axis=1).astype(np.float32)             # [T]
    offs = np.cumsum(c.astype(np.float64))              # [T]

    E = np.exp(transitions.astype(np.float32))
    # col layout: (chain, j, i) blocks of 128 — [E00, E10, E01, E11]
    e_packed = np.empty((128, 2 * KT * KT * 128), np.float16)
    for i in range(KT):
        for j in range(KT):
            blk = E[128 * i:128 * (i + 1), 128 * j:128 * (j + 1)]
            e_packed[:, (j * KT + i) * 128:(j * KT + i + 1) * 128] = blk
            # bwd chain applies E (not E^T): stationary lhsT = E^T tiles
            e_packed[:, (KT * KT + j * KT + i) * 128:
                     (KT * KT + j * KT + i + 1) * 128] = \
                E[128 * j:128 * (j + 1), 128 * i:128 * (i + 1)].T
    Fx = np.exp(f - c[:, None, None]).astype(np.float32)  # [T, B, K]

    Tb = (feats_len - 1).astype(np.int64)                 # [B]
    f_maps = []
    for core in range(N_CORES):
        bg = np.arange(core * BL, (core + 1) * BL)
        fwd = Fx[:tp, bg, :]                              # [tp, BL, K]
        rev = Tb[bg][None, :] - np.arange(tp)[:, None]    # [tp, BL]
        bwd = Fx[np.maximum(rev, 0), bg[None, :], :]      # [tp, BL, K]
        # zero frames past each column's start: dead columns decay to 0
        # instead of overflowing fp16 (they are never read by the host)
        bwd[rev < 0] = 0.0
        # layout [128, tp, chain, j, b]
        arr = np.stack([fwd, bwd], axis=1)                # [tp, 2, BL, K]
        blk = arr.reshape(tp, 2, BL, KT, 128).transpose(4, 0, 1, 3, 2)
        f_maps.append({"f_in": np.ascontiguousarray(
            blk.reshape(128, tp * 2 * GW)).astype(np.float16)})
    return e_packed, f_maps, offs, c, Fx


def _gold_score(feats, transitions, tags, feats_len):
    f = feats.transpose(1, 0, 2).astype(np.float32)       # [T, B, K]
    tg = tags.T.astype(np.int64)                          # [T, B]
    mask = (np.arange(T)[:, None] < feats_len[None, :])
    maskf = mask.astype(np.float32)
    emit = np.take_along_axis(f, tg[:, :, None], axis=2)[:, :, 0] * maskf
    u = emit.sum(axis=0, dtype=np.float32)
    t_mask = maskf[:-1] * maskf[1:]
    t_score = transitions.astype(np.float32)[tg[:-1], tg[1:]] * t_mask
    return (u + t_score.sum(axis=0, dtype=np.float32)).astype(np.float32)


def kernel(feats, transitions, tags, feats_len, _results_hook=None,
           _trace=False):
    from concourse.bass_utils import run_bass_kernel_spmd

    feats = np.asarray(feats, dtype=np.float32)
    transitions = np.asarray(transitions, dtype=np.float32)
    tags_np = np.asarray(tags)
    feats_len_np = np.asarray(feats_len).astype(np.int64)

    # the device loop covers the bulk; the few columns whose meet-point
    # indices land in the last HOST_H periods are stepped forward on the
    # host in float64 (same recurrence, exact frames)
    HOST_H = 2
    max_len = int(feats_len_np.max())
    tp = min(T, max((max_len - 1) // 2 + 1 - HOST_H, 2 * CHUNK))
    if ("nc", tp) not in _cache:
        _cache[("nc", tp)] = _build_nc(tp)
    nc = _cache[("nc", tp)]

    e_packed, f_maps, offs, _c, Fx = _prepare(feats, transitions,
                                              feats_len_np, tp)
    in_maps = [{"e_in": e_packed, **f_maps[core]} for core in range(N_CORES)]

    res = run_bass_kernel_spmd(nc, in_maps, core_ids=list(range(N_CORES)),
                               trace=_trace)
    if _results_hook is not None:
        _results_hook(res)

    u = _gold_score(feats, transitions, tags_np, feats_len_np)
    E64 = np.exp(transitions.astype(np.float64))
    Tb = feats_len_np - 1
    loss = np.empty(B, np.float32)
    for core in range(N_CORES):
        bg = np.arange(core * BL, (core + 1) * BL)
        # [128, tp, j, b] -> [tp, K, b]
        P = res.results[core]["p_out"].reshape(128, tp, KT, BL).astype(
            np.float64).transpose(1, 2, 0, 3).reshape(tp, K, BL)
        S = res.results[core]["q_out"].reshape(128, tp, KT, BL).astype(
            np.float64).transpose(1, 2, 0, 3).reshape(tp, K, BL)
        for i, b in enumerate(bg):
            tb = int(Tb[b])
            if tb == 0:
                d = P[0, :, i].sum()
            else:
                mb = tb // 2
                sidx = tb - mb - 1
                jx = _avail(min(mb, tp - 1), tp)
                x = P[jx, :, i]                       # fwd state at jx
                for t_ in range(jx + 1, mb + 1):
                    x = (E64.T @ x) * Fx[t_, b, :]
                jy = _avail(min(sidx, tp - 1), tp)
                y = S[jy, :, i]                       # bwd state at jy
                for s_ in range(jy + 1, sidx + 1):
                    y = (E64 @ y) * Fx[tb - s_, b, :]
                d = (x @ E64) @ y
            loss[b] = np.float32(np.log(d) + offs[tb] - u[b])
    return loss


def _avail(idx, tp):
    """Largest device-dumped state index <= idx (slots {0,1} mod 4, plus
    the final-period pair)."""
    if idx >= tp - 2:
        return idx
    while idx % 4 > 1:
        idx -= 1
    return idx
